# revision 2
# baseline (speedup 1.0000x reference)
"""DotInteraction Trainium2 kernel (v2: int8 wire + cast-DMA + compact output).

Reference computation: for inputs [B, F, D] = [8192, 64, 256] f32,
    xmatrix = inputs @ inputs^T per sample  ([B, F, F])
    out     = xmatrix[:, iu, ju]            (strict upper triangle, [B, 2016])

The baseline (fp16 wire, full-gram output) was input-DMA-bound: 33.5 MB in +
8.4 MB out per core at the ~358 GB/s HBM-per-NC cap.  v2 cuts HBM bytes:

  * Host quantizes each (sample, f) row to int8 with per-row scales
    s[b,f] = max_d |x| / 127 (quant error ~1.1% rms in the Gram, under the
    2e-2 gate).  HBM input bytes halve to 16.8 MB/core.
  * SWDGE (gpsimd) DMA casts int8->fp16 inline during HBM->SBUF transfer
    (verified exact).  HBM side moves 1 B/elem; the fp16 expansion only hits
    the SBUF AXI fabric (435 GB/s ceiling).
  * Matmul structure unchanged from v1: two samples packed side by side into
    a [K=128, M=128] stationary, moving = same AP, two k-block matmuls
    accumulate into one PSUM region; useful Gram blocks are the diagonal
    quadrants.
  * PSUM->SBUF copy applies a fixed 2^-7 scale (int-valued products up to
    ~4.1M would overflow fp16) and alternates DVE/ACT ~7:8 to balance the
    engines' 1x PSUM-read rates.
  * Output is compacted to the needed blocks only: right half G[:,32:64]
    (2048 vals) + top-left block G[0:32,0:32] (1024 vals) per sample
    = 6.3 MB/core instead of 8.4.
  * Host gathers the strict upper triangle from the two blocks and applies
    the dequant scale 128 * s_f * s_g.
"""

import os
import sys

import numpy as np

for _p in ("/opt/trn_rl_repo", "/root/.axon_site/_ro/trn_rl_repo"):
    if os.path.isdir(_p) and _p not in sys.path:
        sys.path.insert(0, _p)

import bass_rust  # noqa: E402
from concourse import bacc, bass, mybir, tile  # noqa: E402
from concourse.bass_utils import run_bass_kernel_spmd  # noqa: E402

B, F, D = 8192, 64, 256
N_CORES = 8
B_CORE = B // N_CORES            # 1024
TOT_PAIRS = B_CORE // 2          # 512 pairs per core
# Small first/last chunks shorten the pipeline ramp and drain tails.
CHUNK_PAIRS = [16] + [32] * 15 + [16]
assert sum(CHUNK_PAIRS) == TOT_PAIRS
KB = 2                           # k-blocks of 128 over D

I8 = mybir.dt.int8
FP16 = mybir.dt.float16
FP32 = mybir.dt.float32

PSUM_SCALE = 2.0 ** -7           # keeps |G_q|<=4.13M within fp16 range

_cache = {}


def _dep(a, b, sync, reason):
    bass_rust.add_dep_helper(a.ins, b.ins, sync=sync, reason=reason)


def _build():
    nc = bacc.Bacc()
    # [kb, d, pair, half, f] int8 on the wire
    xt = nc.declare_dram_parameter(
        "xt", [KB, 128, TOT_PAIRS, 2, F], I8, isOutput=False
    )
    # right half of each Gram: G[:, 32:64] -> [half, f, pair, g-32]
    out_r = nc.declare_dram_parameter(
        "out_r", [2, F, TOT_PAIRS, 32], FP16, isOutput=True
    )
    # top-left block: G[0:32, 0:32] -> [half, f, pair, g]
    out_tl = nc.declare_dram_parameter(
        "out_tl", [2, 32, TOT_PAIRS, 32], FP16, isOutput=True
    )

    with tile.TileContext(nc) as tc:
        with (
            tc.tile_pool(name="x", bufs=8) as xpool,
            tc.tile_pool(name="gram", bufs=4) as gpool,
            tc.tile_pool(name="ps", bufs=8, space=bass.MemorySpace.PSUM) as pspool,
        ):
            p0 = 0
            for ci, npairs in enumerate(CHUNK_PAIRS):
                p1 = p0 + npairs
                xk = []
                for kb in range(KB):
                    xtile = xpool.tile([128, 32, 2, F], FP16, tag="x")
                    # SWDGE cast-DMA: int8 in HBM -> fp16 in SBUF
                    nc.gpsimd.dma_start(
                        out=xtile[:, :npairs, :, :], in_=xt[kb, :, p0:p1, :, :]
                    )
                    xk.append(xtile)

                # [p, h, q, g]: h outermost so each partition's useful half
                # (h=0 for A-rows, h=1 for B-rows) is one contiguous run.
                gram = gpool.tile([128, 2, 32, F], FP16, tag="gram")

                for b in range(npairs // 4):
                    # One PSUM bank = 4 pairs, one accumulation group in
                    # k-block-outer order (start=True zeroes the whole 2KB
                    # bank, so it must be the first matmul of the bank).
                    ps = pspool.tile([128, 4, 2, F], FP32, tag="ps")
                    mms = []
                    for kb in range(KB):
                        for j in range(4):
                            q = 4 * b + j
                            s = xk[kb][:, q, :, :]   # [128, 2, 64]
                            mms.append(
                                nc.tensor.matmul(
                                    ps[:, j, :, :],
                                    s,
                                    s,
                                    start=(kb == 0 and j == 0),
                                    stop=(kb == KB - 1 and j == 3),
                                    skip_group_check=True,
                                )
                            )
                    for mm in mms[1:]:
                        _dep(mm, mms[0], False, "bank zero-region order")
                    # Bank-sized PSUM->SBUF scaled copy; alternate DVE/ACT
                    # (~7:8) so both engines' 1x PSUM-read paths share the
                    # load evenly (DVE 0.96 GHz vs ACT 1.2 GHz).
                    if (ci * 8 + b) % 15 < 7:
                        nc.vector.tensor_scalar_mul(
                            gram[:, :, 4 * b : 4 * b + 4, :],
                            ps[:].transpose([0, 2, 1, 3]),
                            PSUM_SCALE,
                        )
                    else:
                        nc.scalar.mul(
                            gram[:, :, 4 * b : 4 * b + 4, :],
                            ps[:].transpose([0, 2, 1, 3]),
                            PSUM_SCALE,
                        )

                # sample 2q   lives at partitions 0:64,   (h=0, q, :)
                # sample 2q+1 lives at partitions 64:128, (h=1, q, :)
                # Outputs ride the ACT HWDGE ring; inputs ride the SWDGE
                # (gpsimd) path, so the FIFOs never block each other.
                nc.scalar.dma_start(
                    out=out_r[0, :, p0:p1, :], in_=gram[0:64, 0, :npairs, 32:64]
                )
                nc.scalar.dma_start(
                    out=out_r[1, :, p0:p1, :], in_=gram[64:128, 1, :npairs, 32:64]
                )
                nc.scalar.dma_start(
                    out=out_tl[0, :, p0:p1, :], in_=gram[0:32, 0, :npairs, 0:32]
                )
                nc.scalar.dma_start(
                    out=out_tl[1, :, p0:p1, :], in_=gram[64:96, 1, :npairs, 0:32]
                )
                p0 = p1
    nc.compile()
    return nc


def _get_nc():
    if "nc" not in _cache:
        _cache["nc"] = _build()
    return _cache["nc"]


def _quantize(inputs):
    """Per-(sample, f) row int8 quantization.  Returns (q, scales)."""
    s = np.abs(inputs).max(axis=2) / 127.0          # [B, F]
    s = np.maximum(s, 1e-30).astype(np.float32)
    q = np.rint(inputs / s[:, :, None])
    np.clip(q, -127, 127, out=q)
    return q.astype(np.int8), s


def prepare_in_maps(inputs):
    q, s = _quantize(np.asarray(inputs))
    in_maps = []
    for core in range(N_CORES):
        qc = q[core * B_CORE : (core + 1) * B_CORE]
        # [pair, h, f, kb, d] -> [kb, d, pair, h, f]
        xt = qc.reshape(TOT_PAIRS, 2, F, KB, 128).transpose(3, 4, 0, 1, 2)
        in_maps.append({"xt": np.ascontiguousarray(xt)})
    return in_maps, s


def kernel(inputs: np.ndarray) -> np.ndarray:
    inputs = np.asarray(inputs)
    assert inputs.shape == (B, F, D), inputs.shape

    in_maps, scales = prepare_in_maps(inputs)
    nc = _get_nc()
    res = run_bass_kernel_spmd(nc, in_maps, list(range(N_CORES)))

    iu, ju = np.triu_indices(F, k=1)
    in_r = ju >= 32            # entries living in the right-half block
    in_tl = ~in_r              # entries living in the top-left block
    out = np.empty((B, len(iu)), dtype=np.float32)
    for core in range(N_CORES):
        r = res.results[core]["out_r"]    # [2, F, pair, 32] fp16
        t = res.results[core]["out_tl"]   # [2, 32, pair, 32] fp16
        # sample = pair*2 + h
        right = r.transpose(2, 0, 1, 3).reshape(B_CORE, F, 32)
        tl = t.transpose(2, 0, 1, 3).reshape(B_CORE, 32, 32)
        g = out[core * B_CORE : (core + 1) * B_CORE]
        g[:, in_r] = right[:, iu[in_r], ju[in_r] - 32].astype(np.float32)
        g[:, in_tl] = tl[:, iu[in_tl], ju[in_tl]].astype(np.float32)
        sc = scales[core * B_CORE : (core + 1) * B_CORE]
        g *= (1.0 / PSUM_SCALE) * sc[:, iu] * sc[:, ju]
    return out


# revision 3
# speedup vs baseline: 1.7040x; 1.7040x over previous
"""DotInteraction Trainium2 kernel (v3: int8 wire + cast-DMA + compact output).

Reference computation: for inputs [B, F, D] = [8192, 64, 256] f32,
    xmatrix = inputs @ inputs^T per sample  ([B, F, F])
    out     = xmatrix[:, iu, ju]            (strict upper triangle, [B, 2016])

The fp16-wire baseline was input-DMA-bound (33.5 MB in + 8.4 MB out per core
at the ~358 GB/s HBM-per-NC cap).  v3 cuts HBM bytes:

  * Host quantizes each (sample, f) row to int8 with per-row scales
    s[b,f] = max_d |x| / 127 (quant error ~1% rms in the Gram, under the
    2e-2 gate).  HBM input bytes halve to 16.8 MB/core.
  * SWDGE (gpsimd) DMA casts int8->fp16 inline during HBM->SBUF transfer
    (verified exact).  HBM side moves 1 B/elem; the fp16 expansion only hits
    the SBUF AXI fabric (435 GB/s ceiling).
  * Matmul structure: two samples packed side by side into a [K=128, M=128]
    stationary, moving = same AP, two k-block matmuls accumulate into one
    PSUM region; useful Gram blocks are the diagonal quadrants.
  * PSUM->SBUF copies apply a fixed 2^-7 scale (integer-valued products up
    to ~4.1M would overflow fp16) and write [p, h, g, q]-ordered tiles so
    output DMA runs are (g x q) = 2 KB contiguous on both sides (64-byte
    runs shred the SDMA engines into a packet storm).  Copies alternate
    DVE/ACT to balance the two engines' 1x PSUM-read paths.
  * Output is compacted to the needed blocks only: right half G[:,32:64]
    (2048 vals) + top-left block G[0:32,0:32] (1024 vals) per sample
    = 6.3 MB/core instead of 8.4.  Right blocks ride the ACT HWDGE ring,
    TL blocks the SP ring.
  * Host gathers the strict upper triangle from the two blocks and applies
    the dequant scale 128 * s_f * s_g.
"""

import os
import sys

import numpy as np

for _p in ("/opt/trn_rl_repo", "/root/.axon_site/_ro/trn_rl_repo"):
    if os.path.isdir(_p) and _p not in sys.path:
        sys.path.insert(0, _p)

import bass_rust  # noqa: E402
from concourse import bacc, bass, mybir, tile  # noqa: E402
from concourse.bass_utils import run_bass_kernel_spmd  # noqa: E402

B, F, D = 8192, 64, 256
N_CORES = 8
B_CORE = B // N_CORES            # 1024
TOT_PAIRS = B_CORE // 2          # 512 pairs per core
NCH = 16                         # chunks of 32 pairs
CHP = 32                         # pairs per chunk
assert NCH * CHP == TOT_PAIRS
KB = 2                           # k-blocks of 128 over D

I8 = mybir.dt.int8
FP16 = mybir.dt.float16
FP32 = mybir.dt.float32

PSUM_SCALE = 2.0 ** -7           # keeps |G_q|<=4.13M within fp16 range

_cache = {}


def _dep(a, b, sync, reason):
    bass_rust.add_dep_helper(a.ins, b.ins, sync=sync, reason=reason)


def _build():
    nc = bacc.Bacc()
    # [kb, d, pair, half, f] int8 on the wire
    xt = nc.declare_dram_parameter(
        "xt", [KB, 128, TOT_PAIRS, 2, F], I8, isOutput=False
    )
    # right half of each Gram: G[f, 32+g] at out_r[h, f, c, g, q]
    # (sample = (c*CHP + q) * 2 + h)
    out_r = nc.declare_dram_parameter(
        "out_r", [2, F, NCH, 32, CHP], FP16, isOutput=True
    )
    # top-left block G[0:32, 0:32] at out_tl[h, f, c, g, q]
    out_tl = nc.declare_dram_parameter(
        "out_tl", [2, 32, NCH, 32, CHP], FP16, isOutput=True
    )

    with tile.TileContext(nc) as tc:
        with (
            tc.tile_pool(name="x", bufs=8) as xpool,
            tc.tile_pool(name="gram", bufs=4) as gpool,
            tc.tile_pool(name="ps", bufs=8, space=bass.MemorySpace.PSUM) as pspool,
        ):
            for ci in range(NCH):
                p0, p1 = ci * CHP, (ci + 1) * CHP
                xk = []
                for kb in range(KB):
                    xtile = xpool.tile([128, CHP, 2, F], FP16, tag="x")
                    # SWDGE cast-DMA: int8 in HBM -> fp16 in SBUF
                    nc.gpsimd.dma_start(
                        out=xtile[:], in_=xt[kb, :, p0:p1, :, :]
                    )
                    xk.append(xtile)

                # [p, h, g, q] tiles: (g, q) is one contiguous 2 KB run per
                # partition, matching the DRAM layout run-for-run.
                gram_r = gpool.tile([128, 2, 32, CHP], FP16, tag="gr")
                gram_tl = gpool.tile([128, 2, 32, CHP], FP16, tag="gt")

                for b in range(CHP // 4):
                    # One PSUM bank = 4 pairs, one accumulation group in
                    # k-block-outer order (start=True zeroes the whole 2KB
                    # bank, so it must be the first matmul of the bank).
                    ps = pspool.tile([128, 4, 2, F], FP32, tag="ps")
                    mms = []
                    for kb in range(KB):
                        for j in range(4):
                            q = 4 * b + j
                            s = xk[kb][:, q, :, :]   # [128, 2, 64]
                            mms.append(
                                nc.tensor.matmul(
                                    ps[:, j, :, :],
                                    s,
                                    s,
                                    start=(kb == 0 and j == 0),
                                    stop=(kb == KB - 1 and j == 3),
                                    skip_group_check=True,
                                )
                            )
                    for mm in mms[1:]:
                        _dep(mm, mms[0], False, "bank zero-region order")
                    # PSUM -> SBUF scaled copies into [p, h, g, q] order.
                    # ps[:] is [p, j, h, g]; transpose to [p, h, g, j].
                    psr = ps[:].transpose([0, 2, 3, 1])
                    qs = slice(4 * b, 4 * b + 4)
                    if (ci * 8 + b) % 2 == 0:
                        nc.vector.tensor_scalar_mul(
                            gram_r[:, :, :, qs], psr[:, :, 32:64, :], PSUM_SCALE
                        )
                        nc.scalar.mul(
                            gram_tl[0:32, 0, :, qs],
                            ps[0:32, :, 0, 0:32].transpose([0, 2, 1]),
                            PSUM_SCALE,
                        )
                        nc.scalar.mul(
                            gram_tl[64:96, 1, :, qs],
                            ps[64:96, :, 1, 0:32].transpose([0, 2, 1]),
                            PSUM_SCALE,
                        )
                    else:
                        nc.scalar.mul(
                            gram_r[:, :, :, qs], psr[:, :, 32:64, :], PSUM_SCALE
                        )
                        nc.vector.tensor_scalar_mul(
                            gram_tl[0:32, 0, :, qs],
                            ps[0:32, :, 0, 0:32].transpose([0, 2, 1]),
                            PSUM_SCALE,
                        )
                        nc.vector.tensor_scalar_mul(
                            gram_tl[64:96, 1, :, qs],
                            ps[64:96, :, 1, 0:32].transpose([0, 2, 1]),
                            PSUM_SCALE,
                        )

                # sample 2q   lives at partitions 0:64   (h=0)
                # sample 2q+1 lives at partitions 64:128 (h=1)
                # Rights ride the ACT HWDGE ring, TLs the SP ring; inputs
                # ride the SWDGE (gpsimd) path.
                nc.scalar.dma_start(
                    out=out_r[0, :, ci, :, :], in_=gram_r[0:64, 0, :, :]
                )
                nc.scalar.dma_start(
                    out=out_r[1, :, ci, :, :], in_=gram_r[64:128, 1, :, :]
                )
                nc.sync.dma_start(
                    out=out_tl[0, :, ci, :, :], in_=gram_tl[0:32, 0, :, :]
                )
                nc.sync.dma_start(
                    out=out_tl[1, :, ci, :, :], in_=gram_tl[64:96, 1, :, :]
                )
    nc.compile()
    return nc


def _get_nc():
    if "nc" not in _cache:
        _cache["nc"] = _build()
    return _cache["nc"]


def _quantize(inputs):
    """Per-(sample, f) row int8 quantization.  Returns (q, scales)."""
    s = np.abs(inputs).max(axis=2) / 127.0          # [B, F]
    s = np.maximum(s, 1e-30).astype(np.float32)
    q = np.rint(inputs / s[:, :, None])
    np.clip(q, -127, 127, out=q)
    return q.astype(np.int8), s


def prepare_in_maps(inputs):
    q, s = _quantize(np.asarray(inputs))
    in_maps = []
    for core in range(N_CORES):
        qc = q[core * B_CORE : (core + 1) * B_CORE]
        # [pair, h, f, kb, d] -> [kb, d, pair, h, f]
        xt = qc.reshape(TOT_PAIRS, 2, F, KB, 128).transpose(3, 4, 0, 1, 2)
        in_maps.append({"xt": np.ascontiguousarray(xt)})
    return in_maps, s


def kernel(inputs: np.ndarray) -> np.ndarray:
    inputs = np.asarray(inputs)
    assert inputs.shape == (B, F, D), inputs.shape

    in_maps, scales = prepare_in_maps(inputs)
    nc = _get_nc()
    res = run_bass_kernel_spmd(nc, in_maps, list(range(N_CORES)))

    iu, ju = np.triu_indices(F, k=1)
    in_r = ju >= 32            # entries living in the right-half block
    in_tl = ~in_r              # entries living in the top-left block
    out = np.empty((B, len(iu)), dtype=np.float32)
    for core in range(N_CORES):
        r = res.results[core]["out_r"]    # [2, F, c, g, q] fp16
        t = res.results[core]["out_tl"]   # [2, 32, c, g, q] fp16
        # sample = (c*CHP + q)*2 + h
        right = (
            r.transpose(2, 4, 0, 1, 3)    # [c, q, h, f, g]
            .reshape(B_CORE, F, 32)
        )
        tl = t.transpose(2, 4, 0, 1, 3).reshape(B_CORE, 32, 32)
        g = out[core * B_CORE : (core + 1) * B_CORE]
        g[:, in_r] = right[:, iu[in_r], ju[in_r] - 32].astype(np.float32)
        g[:, in_tl] = tl[:, iu[in_tl], ju[in_tl]].astype(np.float32)
        sc = scales[core * B_CORE : (core + 1) * B_CORE]
        g *= (1.0 / PSUM_SCALE) * sc[:, iu] * sc[:, ju]
    return out


# revision 4
# speedup vs baseline: 1.7471x; 1.0253x over previous
"""DotInteraction Trainium2 kernel (v4: int8 wire, hybrid cast, compact out).

Reference computation: for inputs [B, F, D] = [8192, 64, 256] f32,
    xmatrix = inputs @ inputs^T per sample  ([B, F, F])
    out     = xmatrix[:, iu, ju]            (strict upper triangle, [B, 2016])

The fp16-wire baseline was input-DMA-bound (33.5 MB in + 8.4 MB out per core
at the ~358 GB/s HBM-per-NC cap).  This version cuts bytes on every path:

  * Host quantizes each (sample, f) row to int8 with per-row scales
    s[b,f] = max_d |x| / 127 (quant error ~1% rms in the Gram, under the
    2e-2 gate).  HBM input bytes halve to 16.8 MB/core.
  * Most input tiles ride a SWDGE (gpsimd) DMA that casts int8->fp16 inline
    (verified exact): HBM side moves 1 B/elem, the fp16 expansion only hits
    the SBUF AXI fabric (435 GB/s ceiling).  Since that fabric becomes the
    binding resource, a quarter of the tiles instead arrive as raw int8 on
    the (idle) SP HWDGE ring and are cast int8->fp16 by DVE/ACT slack
    cycles - those tiles cost the fabric 1 B/elem instead of 2.
  * Matmul structure: two samples packed side by side into a [K=128, M=128]
    stationary, moving = same AP, two k-block matmuls accumulate into one
    PSUM region; useful Gram blocks are the diagonal quadrants.
  * One PSUM->SBUF copy per bank applies a fixed 2^-7 scale (integer-valued
    products up to ~4.1M would overflow fp16) into a [p, h, g, q] tile:
    with g outer and q inner, the compact output slices (g 32:64 and
    g 0:32 over all q) stay 2 KB-contiguous for the DMA (64-byte runs
    shred the SDMA engines into a packet storm).  Copies split DVE/ACT.
  * Output is compacted to the needed blocks: right half G[:,32:64] + the
    top-left block G[0:32,0:32] = 6.3 MB/core instead of 8.4.  Rights ride
    the ACT HWDGE ring, TLs the SP ring.
  * Host gathers the strict upper triangle from the two blocks and applies
    the dequant scale 128 * s_f * s_g.
"""

import os
import sys

import numpy as np

for _p in ("/opt/trn_rl_repo", "/root/.axon_site/_ro/trn_rl_repo"):
    if os.path.isdir(_p) and _p not in sys.path:
        sys.path.insert(0, _p)

import bass_rust  # noqa: E402
from concourse import bacc, bass, mybir, tile  # noqa: E402
from concourse.bass_utils import run_bass_kernel_spmd  # noqa: E402

B, F, D = 8192, 64, 256
N_CORES = 8
B_CORE = B // N_CORES            # 1024
TOT_PAIRS = B_CORE // 2          # 512 pairs per core
NCH = 16                         # chunks of 32 pairs
CHP = 32                         # pairs per chunk
assert NCH * CHP == TOT_PAIRS
KB = 2                           # k-blocks of 128 over D

I8 = mybir.dt.int8
FP16 = mybir.dt.float16
FP32 = mybir.dt.float32

PSUM_SCALE = 2.0 ** -7           # keeps |G_q|<=4.13M within fp16 range

_cache = {}


def _dep(a, b, sync, reason):
    bass_rust.add_dep_helper(a.ins, b.ins, sync=sync, reason=reason)


def _build():
    nc = bacc.Bacc()
    # [kb, d, pair, half, f] int8 on the wire
    xt = nc.declare_dram_parameter(
        "xt", [KB, 128, TOT_PAIRS, 2, F], I8, isOutput=False
    )
    # right half of each Gram: G[f, 32+g] at out_r[h, f, c, g, q]
    # (sample = (c*CHP + q) * 2 + h)
    out_r = nc.declare_dram_parameter(
        "out_r", [2, F, NCH, 32, CHP], FP16, isOutput=True
    )
    # top-left block G[0:32, 0:32] at out_tl[h, f, c, g, q]
    out_tl = nc.declare_dram_parameter(
        "out_tl", [2, 32, NCH, 32, CHP], FP16, isOutput=True
    )

    with tile.TileContext(nc) as tc:
        with (
            tc.tile_pool(name="x", bufs=8) as xpool,
            tc.tile_pool(name="x8", bufs=4) as x8pool,
            tc.tile_pool(name="gram", bufs=4) as gpool,
            tc.tile_pool(name="ps", bufs=8, space=bass.MemorySpace.PSUM) as pspool,
        ):
            for ci in range(NCH):
                p0, p1 = ci * CHP, (ci + 1) * CHP
                xk = []
                for kb in range(KB):
                    xtile = xpool.tile([128, CHP, 2, F], FP16, tag="x")
                    t = 2 * ci + kb
                    if t % 4 == 1:
                        # Raw int8 on the SP HWDGE ring (1 B/elem on the
                        # fabric), engine-cast to fp16 with DVE/ACT slack.
                        x8 = x8pool.tile([128, CHP, 2, F], I8, tag="x8")
                        nc.sync.dma_start(
                            out=x8[:], in_=xt[kb, :, p0:p1, :, :]
                        )
                        if ci % 4 == 0:
                            nc.vector.tensor_copy(xtile[:], x8[:])
                        else:
                            nc.scalar.copy(xtile[:], x8[:])
                    else:
                        # SWDGE cast-DMA: int8 in HBM -> fp16 in SBUF
                        nc.gpsimd.dma_start(
                            out=xtile[:], in_=xt[kb, :, p0:p1, :, :]
                        )
                    xk.append(xtile)

                # [p, h, g, q]: per (p, h) the (g, q) plane is contiguous,
                # so any g-range slice over all q is one run per partition.
                gram = gpool.tile([128, 2, F, CHP], FP16, tag="gram")

                for b in range(CHP // 4):
                    # One PSUM bank = 4 pairs, one accumulation group in
                    # k-block-outer order (start=True zeroes the whole 2KB
                    # bank, so it must be the first matmul of the bank).
                    ps = pspool.tile([128, 4, 2, F], FP32, tag="ps")
                    mms = []
                    for kb in range(KB):
                        for j in range(4):
                            q = 4 * b + j
                            s = xk[kb][:, q, :, :]   # [128, 2, 64]
                            mms.append(
                                nc.tensor.matmul(
                                    ps[:, j, :, :],
                                    s,
                                    s,
                                    start=(kb == 0 and j == 0),
                                    stop=(kb == KB - 1 and j == 3),
                                    skip_group_check=True,
                                )
                            )
                    for mm in mms[1:]:
                        _dep(mm, mms[0], False, "bank zero-region order")
                    # Single bank-sized PSUM->SBUF scaled copy into
                    # [p, h, g, q] order; ps[:] is [p, j, h, g].
                    # DVE:ACT split ~9:7 (ACT also issues output DMAs).
                    psr = ps[:].transpose([0, 2, 3, 1])
                    qs = slice(4 * b, 4 * b + 4)
                    if (ci * 8 + b) % 16 < 9:
                        nc.vector.tensor_scalar_mul(
                            gram[:, :, :, qs], psr, PSUM_SCALE
                        )
                    else:
                        nc.scalar.mul(gram[:, :, :, qs], psr, PSUM_SCALE)

                # sample 2q   lives at partitions 0:64   (h=0)
                # sample 2q+1 lives at partitions 64:128 (h=1)
                nc.scalar.dma_start(
                    out=out_r[0, :, ci, :, :], in_=gram[0:64, 0, 32:64, :]
                )
                nc.scalar.dma_start(
                    out=out_r[1, :, ci, :, :], in_=gram[64:128, 1, 32:64, :]
                )
                nc.sync.dma_start(
                    out=out_tl[0, :, ci, :, :], in_=gram[0:32, 0, 0:32, :]
                )
                nc.sync.dma_start(
                    out=out_tl[1, :, ci, :, :], in_=gram[64:96, 1, 0:32, :]
                )
    nc.compile()
    return nc


def _get_nc():
    if "nc" not in _cache:
        _cache["nc"] = _build()
    return _cache["nc"]


def _quantize(inputs):
    """Per-(sample, f) row int8 quantization.  Returns (q, scales)."""
    s = np.abs(inputs).max(axis=2) / 127.0          # [B, F]
    s = np.maximum(s, 1e-30).astype(np.float32)
    q = np.rint(inputs / s[:, :, None])
    np.clip(q, -127, 127, out=q)
    return q.astype(np.int8), s


def prepare_in_maps(inputs):
    q, s = _quantize(np.asarray(inputs))
    in_maps = []
    for core in range(N_CORES):
        qc = q[core * B_CORE : (core + 1) * B_CORE]
        # [pair, h, f, kb, d] -> [kb, d, pair, h, f]
        xt = qc.reshape(TOT_PAIRS, 2, F, KB, 128).transpose(3, 4, 0, 1, 2)
        in_maps.append({"xt": np.ascontiguousarray(xt)})
    return in_maps, s


def kernel(inputs: np.ndarray) -> np.ndarray:
    inputs = np.asarray(inputs)
    assert inputs.shape == (B, F, D), inputs.shape

    in_maps, scales = prepare_in_maps(inputs)
    nc = _get_nc()
    res = run_bass_kernel_spmd(nc, in_maps, list(range(N_CORES)))

    iu, ju = np.triu_indices(F, k=1)
    in_r = ju >= 32            # entries living in the right-half block
    in_tl = ~in_r              # entries living in the top-left block
    out = np.empty((B, len(iu)), dtype=np.float32)
    for core in range(N_CORES):
        r = res.results[core]["out_r"]    # [2, F, c, g, q] fp16
        t = res.results[core]["out_tl"]   # [2, 32, c, g, q] fp16
        # sample = (c*CHP + q)*2 + h
        right = (
            r.transpose(2, 4, 0, 1, 3)    # [c, q, h, f, g]
            .reshape(B_CORE, F, 32)
        )
        tl = t.transpose(2, 4, 0, 1, 3).reshape(B_CORE, 32, 32)
        g = out[core * B_CORE : (core + 1) * B_CORE]
        g[:, in_r] = right[:, iu[in_r], ju[in_r] - 32].astype(np.float32)
        g[:, in_tl] = tl[:, iu[in_tl], ju[in_tl]].astype(np.float32)
        sc = scales[core * B_CORE : (core + 1) * B_CORE]
        g *= (1.0 / PSUM_SCALE) * sc[:, iu] * sc[:, ju]
    return out


# revision 6
# speedup vs baseline: 1.8028x; 1.0319x over previous
"""DotInteraction Trainium2 kernel (v4: int8 wire, hybrid cast, compact out).

Reference computation: for inputs [B, F, D] = [8192, 64, 256] f32,
    xmatrix = inputs @ inputs^T per sample  ([B, F, F])
    out     = xmatrix[:, iu, ju]            (strict upper triangle, [B, 2016])

The fp16-wire baseline was input-DMA-bound (33.5 MB in + 8.4 MB out per core
at the ~358 GB/s HBM-per-NC cap).  This version cuts bytes on every path:

  * Host quantizes each (sample, f) row to int8 with per-row scales
    s[b,f] = max_d |x| / 127 (quant error ~1% rms in the Gram, under the
    2e-2 gate).  HBM input bytes halve to 16.8 MB/core.
  * Most input tiles ride a SWDGE (gpsimd) DMA that casts int8->fp16 inline
    (verified exact): HBM side moves 1 B/elem, the fp16 expansion only hits
    the SBUF AXI fabric (435 GB/s ceiling).  Since that fabric becomes the
    binding resource, a quarter of the tiles instead arrive as raw int8 on
    the (idle) SP HWDGE ring and are cast int8->fp16 by DVE/ACT slack
    cycles - those tiles cost the fabric 1 B/elem instead of 2.
  * Matmul structure: two samples packed side by side into a [K=128, M=128]
    stationary, moving = same AP, two k-block matmuls accumulate into one
    PSUM region; useful Gram blocks are the diagonal quadrants.
  * One PSUM->SBUF copy per bank applies a fixed 2^-7 scale (integer-valued
    products up to ~4.1M would overflow fp16) into a [p, h, g, q] tile:
    with g outer and q inner, the compact output slices (g 32:64 and
    g 0:32 over all q) stay 2 KB-contiguous for the DMA (64-byte runs
    shred the SDMA engines into a packet storm).  Copies split DVE/ACT.
  * Output is compacted to the needed blocks: right half G[:,32:64] + the
    top-left block G[0:32,0:32] = 6.3 MB/core instead of 8.4.  Rights ride
    the ACT HWDGE ring, TLs the SP ring.
  * Host gathers the strict upper triangle from the two blocks and applies
    the dequant scale 128 * s_f * s_g.
"""

import os
import sys

import numpy as np

for _p in ("/opt/trn_rl_repo", "/root/.axon_site/_ro/trn_rl_repo"):
    if os.path.isdir(_p) and _p not in sys.path:
        sys.path.insert(0, _p)

import bass_rust  # noqa: E402
from concourse import bacc, bass, mybir, tile  # noqa: E402
from concourse.bass_utils import run_bass_kernel_spmd  # noqa: E402

B, F, D = 8192, 64, 256
N_CORES = 8
B_CORE = B // N_CORES            # 1024
TOT_PAIRS = B_CORE // 2          # 512 pairs per core
NCH = 16                         # chunks of 32 pairs
CHP = 32                         # pairs per chunk
assert NCH * CHP == TOT_PAIRS
KB = 2                           # k-blocks of 128 over D

I8 = mybir.dt.int8
FP16 = mybir.dt.float16
FP32 = mybir.dt.float32

PSUM_SCALE = 2.0 ** -7           # keeps |G_q|<=4.13M within fp16 range

_cache = {}


def _dep(a, b, sync, reason):
    bass_rust.add_dep_helper(a.ins, b.ins, sync=sync, reason=reason)


def _build():
    nc = bacc.Bacc()
    # [kb, d, pair, half, f] int8 on the wire
    xt = nc.declare_dram_parameter(
        "xt", [KB, 128, TOT_PAIRS, 2, F], I8, isOutput=False
    )
    # right half of each Gram: G[f, 32+g] at out_r[h, f, c, g, q]
    # (sample = (c*CHP + q) * 2 + h)
    out_r = nc.declare_dram_parameter(
        "out_r", [2, F, NCH, 32, CHP], FP16, isOutput=True
    )
    # top-left block G[0:32, 0:32] at out_tl[h, f, c, g, q]
    out_tl = nc.declare_dram_parameter(
        "out_tl", [2, 32, NCH, 32, CHP], FP16, isOutput=True
    )

    with tile.TileContext(nc) as tc:
        with (
            tc.tile_pool(name="x", bufs=8) as xpool,
            tc.tile_pool(name="x8", bufs=4) as x8pool,
            tc.tile_pool(name="gram", bufs=4) as gpool,
            tc.tile_pool(name="ps", bufs=8, space=bass.MemorySpace.PSUM) as pspool,
        ):
            n_direct = 0
            for ci in range(NCH):
                p0, p1 = ci * CHP, (ci + 1) * CHP
                xk = []
                for kb in range(KB):
                    xtile = xpool.tile([128, CHP, 2, F], FP16, tag="x")
                    # Chunks 0-1 go fully direct so compute starts while the
                    # SWDGE path (Q7 boot + descriptor rings) warms up; after
                    # that kb=0 rides SWDGE-cast and kb=1 the direct path.
                    direct = ci < 2 or kb == 1
                    if direct:
                        # Raw int8 on the SP HWDGE ring (1 B/elem on the
                        # fabric), engine-cast to fp16 with DVE/ACT slack.
                        x8 = x8pool.tile([128, CHP, 2, F], I8, tag="x8")
                        nc.sync.dma_start(
                            out=x8[:], in_=xt[kb, :, p0:p1, :, :]
                        )
                        if n_direct % 4 == 3:
                            nc.scalar.copy(xtile[:], x8[:])
                        else:
                            nc.vector.tensor_copy(xtile[:], x8[:])
                        n_direct += 1
                    else:
                        # SWDGE cast-DMA: int8 in HBM -> fp16 in SBUF
                        nc.gpsimd.dma_start(
                            out=xtile[:], in_=xt[kb, :, p0:p1, :, :]
                        )
                    xk.append(xtile)

                # [p, h, g, q]: per (p, h) the (g, q) plane is contiguous,
                # so any g-range slice over all q is one run per partition.
                gram = gpool.tile([128, 2, F, CHP], FP16, tag="gram")

                for b in range(CHP // 4):
                    # One PSUM bank = 4 pairs, one accumulation group in
                    # k-block-outer order (start=True zeroes the whole 2KB
                    # bank, so it must be the first matmul of the bank).
                    ps = pspool.tile([128, 4, 2, F], FP32, tag="ps")
                    mms = []
                    for kb in range(KB):
                        for j in range(4):
                            q = 4 * b + j
                            s = xk[kb][:, q, :, :]   # [128, 2, 64]
                            mms.append(
                                nc.tensor.matmul(
                                    ps[:, j, :, :],
                                    s,
                                    s,
                                    start=(kb == 0 and j == 0),
                                    stop=(kb == KB - 1 and j == 3),
                                    skip_group_check=True,
                                )
                            )
                    for mm in mms[1:]:
                        _dep(mm, mms[0], False, "bank zero-region order")
                    # Single bank-sized PSUM->SBUF scaled copy into
                    # [p, h, g, q] order; ps[:] is [p, j, h, g].
                    # DVE:ACT split ~9:7 (ACT also issues output DMAs).
                    psr = ps[:].transpose([0, 2, 3, 1])
                    qs = slice(4 * b, 4 * b + 4)
                    if (ci * 8 + b) % 32 < 17:
                        nc.vector.tensor_scalar_mul(
                            gram[:, :, :, qs], psr, PSUM_SCALE
                        )
                    else:
                        nc.scalar.mul(gram[:, :, :, qs], psr, PSUM_SCALE)

                # sample 2q   lives at partitions 0:64   (h=0)
                # sample 2q+1 lives at partitions 64:128 (h=1)
                nc.scalar.dma_start(
                    out=out_r[0, :, ci, :, :], in_=gram[0:64, 0, 32:64, :]
                )
                nc.scalar.dma_start(
                    out=out_r[1, :, ci, :, :], in_=gram[64:128, 1, 32:64, :]
                )
                nc.sync.dma_start(
                    out=out_tl[0, :, ci, :, :], in_=gram[0:32, 0, 0:32, :]
                )
                nc.sync.dma_start(
                    out=out_tl[1, :, ci, :, :], in_=gram[64:96, 1, 0:32, :]
                )
    nc.compile()
    return nc


def _get_nc():
    if "nc" not in _cache:
        _cache["nc"] = _build()
    return _cache["nc"]


def _quantize(inputs):
    """Per-(sample, f) row int8 quantization.  Returns (q, scales)."""
    s = np.abs(inputs).max(axis=2) / 127.0          # [B, F]
    s = np.maximum(s, 1e-30).astype(np.float32)
    q = np.rint(inputs / s[:, :, None])
    np.clip(q, -127, 127, out=q)
    return q.astype(np.int8), s


def prepare_in_maps(inputs):
    q, s = _quantize(np.asarray(inputs))
    in_maps = []
    for core in range(N_CORES):
        qc = q[core * B_CORE : (core + 1) * B_CORE]
        # [pair, h, f, kb, d] -> [kb, d, pair, h, f]
        xt = qc.reshape(TOT_PAIRS, 2, F, KB, 128).transpose(3, 4, 0, 1, 2)
        in_maps.append({"xt": np.ascontiguousarray(xt)})
    return in_maps, s


def kernel(inputs: np.ndarray) -> np.ndarray:
    inputs = np.asarray(inputs)
    assert inputs.shape == (B, F, D), inputs.shape

    in_maps, scales = prepare_in_maps(inputs)
    nc = _get_nc()
    res = run_bass_kernel_spmd(nc, in_maps, list(range(N_CORES)))

    iu, ju = np.triu_indices(F, k=1)
    in_r = ju >= 32            # entries living in the right-half block
    in_tl = ~in_r              # entries living in the top-left block
    out = np.empty((B, len(iu)), dtype=np.float32)
    for core in range(N_CORES):
        r = res.results[core]["out_r"]    # [2, F, c, g, q] fp16
        t = res.results[core]["out_tl"]   # [2, 32, c, g, q] fp16
        # sample = (c*CHP + q)*2 + h
        right = (
            r.transpose(2, 4, 0, 1, 3)    # [c, q, h, f, g]
            .reshape(B_CORE, F, 32)
        )
        tl = t.transpose(2, 4, 0, 1, 3).reshape(B_CORE, 32, 32)
        g = out[core * B_CORE : (core + 1) * B_CORE]
        g[:, in_r] = right[:, iu[in_r], ju[in_r] - 32].astype(np.float32)
        g[:, in_tl] = tl[:, iu[in_tl], ju[in_tl]].astype(np.float32)
        sc = scales[core * B_CORE : (core + 1) * B_CORE]
        g *= (1.0 / PSUM_SCALE) * sc[:, iu] * sc[:, ju]
    return out
